# revision 15
# baseline (speedup 1.0000x reference)
"""Trainium2 Bass kernel for CoreSageLayer (GNN mean-aggregate + 3-way linear).

Computation (reference):
    mask = (adj == 1)                      # [N, N] 0/1
    deg  = mask.sum(axis=1)                # [N]
    x1   = (mask @ x) / deg[:, None]       # [N, F]
    out[k] = concat([x1, x], 1) @ W[k] + bias   # [3, N, O]

Distribution: row-shard adj / x1 / out over nodes across 8 cores; replicate
x and weights; no collectives (rows independent).

Device schedule per core (nodes NB=1024, 8 node-tiles of 128):
  stage 1 (per node-tile j): fp8e4m3 DoubleRow matmuls — each instruction
      contracts a 256-neighbor chunk-pair (2 fp8 rows per PE cell, 0.5
      cycles per output column). lhsT = maskT chunk [128, 2, 128], rhs =
      [1 | x] fp8 chunk. Column 0 of the moving tensor is the ones column,
      so PSUM col 0 accumulates the exact degree (0/1 products in fp32).
      Output columns split 129 + 128 across two PSUM tiles because the
      DoubleRow moving limit is 512 (2*257 = 514 would exceed it).
  finalize j: rec = 1/deg (DVE), x1 = psum * rec -> bf16, PE-transpose x1
      into x1T (bf16), then stage 2: out[k] = [x1 | x] @ W[k] as bf16
      matmuls with W for k=0,1 fused into one 512-wide moving tensor.
      Results are written back as bf16 (cast to f32 on host).
"""

import sys

sys.path.insert(0, "/opt/trn_rl_repo")

import numpy as np

N = 8192
F = 256
O = 256
NCORES = 8
NB = N // NCORES          # nodes per core (1024)
JT = NB // 128            # node tiles per core (8)
MCHUNKS = N // 128        # contraction chunks of 128 (64)
CP = MCHUNKS // 2         # chunk-pairs of 256 for DoubleRow (32)
FP = F + 1                # ones column + x columns (257)


def _patch_tile_drain():
    """This container's walrus allows only one sync-wait per CTRL instruction;
    split the Tile kernel-tail drain's waits onto single-wait no-fuse NoOps."""
    import concourse.tile as tile
    from concourse import mybir
    from concourse.tile import ScopedClock

    if getattr(tile.TileContext, "_drain_split_patched", False):
        return

    def _drain_and_barrier(self, tick_clock, wait_clock):
        nc = self.nc
        drain_inst = nc.sync.drain()
        wait_clock.add_sem_waits(
            drain_inst.ins, ScopedClock({None: tick_clock.global_clock})
        )
        si = drain_inst.ins.sync_info
        if si is not None and len(si.on_wait) > 1:
            waits = list(si.on_wait)
            drain_inst.ins.sync_info = mybir.SyncInfo(
                on_wait=[waits[0]], on_update=list(si.on_update)
            )
            for w in waits[1:]:
                nop = nc.sync.nop(nofuse=True, hint="split_wait")
                nop.ins.sync_info = mybir.SyncInfo(on_wait=[w], on_update=[])
        nc.all_engine_barrier()
        assert self.sems is not None
        popped = nc._tile_sem_poison_stack.pop()
        assert popped is self._sem_poison
        nc.clear_and_free_semaphores(list(self.sems.allocated().values()))
        nc.all_engine_barrier()

    tile.TileContext._drain_and_barrier = _drain_and_barrier
    tile.TileContext._drain_split_patched = True

    # Same walrus limitation, general case: any instruction that Tile gave
    # >1 sem-wait (e.g. a DMA with both RAW and WAR deps) fails codegen.
    # Split surplus waits onto fresh single-wait NoOps emitted just before
    # the instruction on the same engine, at the serialized-BIR level.
    import concourse.bass as bass
    import orjson

    _orig_to_json_bytes = bass.Bass.to_json_bytes

    def _to_json_bytes_split(self):
        m = orjson.loads(_orig_to_json_bytes(self))
        ctr = 0
        for fn in m.get("functions", []):
            for bb in fn.get("blocks", []):
                insts = bb.get("instructions", [])
                # Dedupe redundant PE weight loads: legalization emits one
                # Ldweights per Matmult, but consecutive matmuls that share
                # a stationary tensor (the two output-column groups per mask
                # chunk-pair; the three k's per stage-2 f-chunk) only need
                # the first — the PE array keeps weights across matmuls.
                # A duplicate with sync waits/updates becomes a NoOp that
                # preserves them; a bare one is dropped.
                deduped = []
                cur_key = None
                for inst in insts:
                    if inst.get("engine") != "PE":
                        deduped.append(inst)
                        continue
                    op = inst.get("opcode")
                    if op == "Ldweights":
                        key = orjson.dumps([
                            inst.get("ins"), inst.get("perf_mode"),
                            inst.get("is_transpose"),
                            inst.get("tile_position"), inst.get("tile_size"),
                        ])
                        if key == cur_key:
                            si = inst.get("sync_info")
                            if si and (si.get("on_wait") or si.get("on_update")):
                                deduped.append({
                                    "name": inst["name"] + "-LDWNOP",
                                    "opcode": "NoOp",
                                    "engine": "PE",
                                    "ins": [],
                                    "outs": [],
                                    "sync_info": si,
                                })
                            continue
                        cur_key = key
                    elif op != "Matmult":
                        cur_key = None
                    deduped.append(inst)
                insts = deduped
                new = []
                for inst in insts:
                    si = inst.get("sync_info")
                    waits = (si or {}).get("on_wait") or []
                    if len(waits) > 1:
                        for w in waits[:-1]:
                            ctr += 1
                            new.append({
                                "name": f"SWNOP-{ctr}",
                                "opcode": "NoOp",
                                "engine": inst["engine"],
                                "ins": [],
                                "outs": [],
                                "sync_info": {"on_wait": [w], "on_update": []},
                            })
                        si["on_wait"] = [waits[-1]]
                    new.append(inst)
                bb["instructions"] = new
        return orjson.dumps(m)

    bass.Bass.to_json_bytes = _to_json_bytes_split



def build_bass(with_bias: bool):
    import concourse.bass as bass
    import concourse.tile as tile
    from concourse import mybir
    from concourse.masks import make_identity

    _patch_tile_drain()

    fp8 = mybir.dt.float8e4
    bf16 = mybir.dt.bfloat16
    f32 = mybir.dt.float32
    DR = mybir.MatmulPerfMode.DoubleRow

    nc = bass.Bass()
    maskt = nc.dram_tensor("maskt", [JT, 128, CP * 128 * 2], fp8,
                           kind="ExternalInput")
    xp = nc.dram_tensor("xp", [128, CP * FP * 2], fp8, kind="ExternalInput")
    xt = nc.dram_tensor("xt", [F // 128, 128, NB], bf16, kind="ExternalInput")
    # x1-part weights (f < 256) as fp8 pairs for the DoubleRow contraction;
    # x-part weights (f >= 256) in bf16, k=0,1 fused along the output dim
    w1f8 = nc.dram_tensor("w1f8", [128, 3 * O * 2], fp8, kind="ExternalInput")
    w01x = nc.dram_tensor("w01x", [2, 128, 2 * O], bf16, kind="ExternalInput")
    w2x = nc.dram_tensor("w2x", [2, 128, O], bf16, kind="ExternalInput")
    if with_bias:
        biasr = nc.dram_tensor("biasr", [128, O], f32, kind="ExternalInput")
    out = nc.dram_tensor("out", [JT, 128, 3 * O], bf16, kind="ExternalOutput")

    FCH = 2 * F // 128  # 4 f-chunks of 128 in the stage-2 contraction

    with tile.TileContext(nc) as tc:
        with (
            tc.tile_pool(name="const", bufs=1) as const_pool,
            tc.tile_pool(name="mask", bufs=8) as mask_pool,
            tc.tile_pool(name="work", bufs=3) as work_pool,
            tc.tile_pool(name="psumA", bufs=2, space="PSUM") as psumA_pool,
            tc.tile_pool(name="psumB", bufs=2, space="PSUM") as psumB_pool,
            tc.tile_pool(name="psumt", bufs=2, space="PSUM") as psumt_pool,
            tc.tile_pool(name="psum2", bufs=2, space="PSUM") as psum2_pool,
        ):
            # ---- DMA plan: three HWDGE queues.
            #   sync   : mask j0 (pieces), j1, j2, j4, j6
            #   scalar : stage-2 constants, then mask j3, j5, j7
            #   gpsimd : [1|x] pieces, then per-tile output writes
            # First pieces are small so the PE unblocks right after the
            # queues come up.
            mt0 = mask_pool.tile([128, CP, 2, 128], fp8, tag="mt", name="mt0")
            xp_sb = const_pool.tile([128, CP, FP, 2], fp8)
            # Head race: per-queue HWDGE throughput is ~150-180 GB/s, well
            # below the ~400+ GB/s three-queue aggregate. Everything the PE
            # needs in the first ~20us (xp, mask j0/j1, stage-2 consts) is
            # cut into pieces and round-robined across all three queues in
            # deadline order.
            _rr = [nc.sync, nc.gpsimd, nc.scalar]
            _rrn = [0]

            def rr_dma(dst, src):
                q = _rr[_rrn[0] % 3]
                _rrn[0] += 1
                q.dma_start(dst, src)

            # stage-2 constants are allocated up front: the x-part
            # precompute (which fills PE stall bubbles during the head DMA
            # race) needs xT and the x-part weights within the first ~15us.
            w1f8_sb = const_pool.tile([128, 3 * O, 2], fp8)
            xt_sb = [const_pool.tile([128, NB], bf16, tag=f"xt{h}", name=f"xt{h}")
                     for h in range(2)]
            w01x_sb = [const_pool.tile([128, 2 * O], bf16, tag=f"w01x_{fc}",
                                       name=f"w01x_{fc}") for fc in range(2)]
            w2x_sb = [const_pool.tile([128, O], bf16, tag=f"w2x_{fc}",
                                      name=f"w2x_{fc}") for fc in range(2)]

            PIECES = [1, 1, 2, 2, 2, 4, 4, 8, 8]
            offs = []
            off = 0
            for w in PIECES:
                offs.append((off, w))
                off += w
            for n, (off, w) in enumerate(offs):
                rr_dma(xp_sb[:, off:off + w, :, :],
                       xp[:, off * FP * 2:(off + w) * FP * 2])
                rr_dma(mt0[:, off:off + w, :, :],
                       maskt[0, :, off * 256:(off + w) * 256])
                if n == 4:
                    rr_dma(xt_sb[0][:], xt[0])
                elif n == 5:
                    rr_dma(xt_sb[1][:], xt[1])
                    rr_dma(w01x_sb[0][:], w01x[0])
                elif n == 6:
                    rr_dma(w01x_sb[1][:], w01x[1])
                    rr_dma(w2x_sb[0][:], w2x[0])
                    rr_dma(w2x_sb[1][:], w2x[1])
            rr_dma(w1f8_sb[:], w1f8[:])

            def stage1(j, mt, mids):
                psA = psumA_pool.tile([128, 129], f32, tag="psA")
                psB = psumB_pool.tile([128, 128], f32, tag="psB")
                for c in range(CP):
                    if c in mids:
                        mids[c]()
                    lhsT = mt[:, c, :, :]
                    nc.tensor.matmul(
                        psA[:], lhsT,
                        xp_sb[:, c, 0:129, :].rearrange("p f i -> p i f"),
                        start=(c == 0), stop=(c == CP - 1), perf_mode=DR,
                    )
                    nc.tensor.matmul(
                        psB[:], lhsT,
                        xp_sb[:, c, 129:257, :].rearrange("p f i -> p i f"),
                        start=(c == 0), stop=(c == CP - 1), perf_mode=DR,
                    )
                return psA, psB

            def pre_xpart(j):
                jcols = slice(j * 128, (j + 1) * 128)
                pa = psum2_pool.tile([128, 2 * O], f32, tag="po")
                pb = psum2_pool.tile([128, 2 * O], f32, tag="po")
                for fc in range(2):
                    nc.tensor.matmul(
                        pa[:], xt_sb[fc][:, jcols], w01x_sb[fc][:],
                        start=(fc == 0), stop=(fc == 1),
                    )
                    nc.tensor.matmul(
                        pb[:, 0:O], xt_sb[fc][:, jcols], w2x_sb[fc][:],
                        start=(fc == 0), stop=(fc == 1),
                    )
                nc.vector.tensor_copy(xpart_sb[j][:, 0:2 * O], pa[:])
                nc.vector.tensor_copy(xpart_sb[j][:, 2 * O:], pb[:, 0:O])

            # per-tile x-part partials (64*x @ W2), filled by the head
            # precompute, consumed by finalize
            xpart_sb = [const_pool.tile([128, 3 * O], bf16, tag=f"xpart{j}",
                                        name=f"xpart{j}") for j in range(JT)]

            import functools as _ft
            ps0 = stage1(0, mt0, {
                6 + 4 * jj: _ft.partial(lambda j: pre_xpart(j), jj)
                for jj in range(6)
            })

            if with_bias:
                bias_sb = const_pool.tile([128, O], f32)
                rr_dma(bias_sb[:], biasr[:])
            identity = const_pool.tile([128, 128], bf16)
            make_identity(nc, identity)
            # stage-2 runs on 64*x1 / 64*x (keeps x1 out of fp8's subnormal
            # range); the output copy folds the 1/64 back in
            inv64 = const_pool.tile([128, 1], f32)
            nc.gpsimd.memset(inv64[:], 1.0 / 64.0)
            # x1T planes: x1t_sb[p, i, n] = x1[n, i*128 + p], fp8 for the
            # DoubleRow stage-2 contraction (stationary planes separated)
            x1t_sb = const_pool.tile([128, 2, NB], fp8)
            def load_mask(j):
                mt = mask_pool.tile([128, CP, 2, 128], fp8, tag="mt",
                                    name=f"mt{j}")
                npieces = 4 if j <= 3 else 2
                w = CP // npieces
                for qn in range(npieces):
                    rr_dma(mt[:, qn * w:(qn + 1) * w, :, :],
                           maskt[j, :, qn * w * 256:(qn + 1) * w * 256])
                return mt

            def fin_a(j, psA, psB):
                # 1/deg scale + transpose x1 into fp8 planes
                jcols = slice(j * 128, (j + 1) * 128)
                rec = work_pool.tile([128, 1], f32, tag="rec")
                nc.vector.reciprocal(rec[:], psA[:, 0:1])
                x1h = [work_pool.tile([128, 128], bf16, tag=f"x1h{h}",
                                      name=f"x1h{h}") for h in range(2)]
                nc.vector.tensor_scalar_mul(x1h[0][:], psA[:, 1:129], rec[:])
                nc.vector.tensor_scalar_mul(x1h[1][:], psB[:], rec[:])
                for h in range(2):
                    pt = psumt_pool.tile([128, 128], bf16, tag="pt")
                    nc.tensor.transpose(pt[:], x1h[h][:], identity[:])
                    nc.vector.tensor_copy(x1t_sb[:, h, jcols], pt[:])

            def fin_b(j):
                # stage 2: out[k] = x1 @ W1[k] (fp8 DoubleRow, one weight
                # load for all three k) + precomputed x-part + bias.
                # PSUM start-flag zeroing is bank-granular on this HW: the
                # first matmul per bank opens it, the second accumulates.
                jcols = slice(j * 128, (j + 1) * 128)
                po01 = psum2_pool.tile([128, 2 * O], f32, tag="po")
                po2 = psum2_pool.tile([128, 2 * O], f32, tag="po")
                lhsDR = x1t_sb[:, :, jcols]

                def w1k(k):
                    return w1f8_sb[:, k * O:(k + 1) * O, :].rearrange(
                        "p o i -> p i o")

                nc.tensor.matmul(po01[:, 0:O], lhsDR, w1k(0),
                                 start=True, stop=False, perf_mode=DR,
                                 skip_group_check=True)
                nc.tensor.matmul(po01[:, O:2 * O], lhsDR, w1k(1),
                                 start=False, stop=True, perf_mode=DR,
                                 skip_group_check=True)
                nc.tensor.matmul(po2[:, 0:O], lhsDR, w1k(2),
                                 start=True, stop=True, perf_mode=DR,
                                 skip_group_check=True)
                t01 = work_pool.tile([128, 2 * O], f32, tag="t01")
                t2 = work_pool.tile([128, O], f32, tag="t2")
                nc.vector.tensor_add(t01[:], po01[:], xpart_sb[j][:, 0:2 * O])
                nc.vector.tensor_add(t2[:], po2[:, 0:O], xpart_sb[j][:, 2 * O:])
                ot = work_pool.tile([128, 3 * O], bf16, tag="ot")
                if with_bias:
                    sc = work_pool.tile([128, 3 * O], f32, tag="sc")
                    nc.vector.tensor_scalar_mul(sc[:, 0:2 * O], t01[:],
                                                inv64[:])
                    nc.vector.tensor_scalar_mul(sc[:, 2 * O:], t2[:], inv64[:])
                    nc.vector.tensor_add(ot[:, 0:O], sc[:, 0:O], bias_sb[:])
                    nc.vector.tensor_add(ot[:, O:2 * O], sc[:, O:2 * O],
                                         bias_sb[:])
                    nc.vector.tensor_add(ot[:, 2 * O:], sc[:, 2 * O:],
                                         bias_sb[:])
                else:
                    nc.vector.tensor_scalar_mul(ot[:, 0:2 * O], t01[:],
                                                inv64[:])
                    nc.vector.tensor_scalar_mul(ot[:, 2 * O:], t2[:], inv64[:])
                nc.gpsimd.dma_start(out[j], ot[:])

            # software-pipeline by one node-tile: the previous tile's
            # finalize work is injected into this tile's stage-1 chunk loop
            # (transposes at chunk 2, stage-2 at chunk 8) so the PE never
            # stalls on the DVE chain and the kernel tail stays short.
            import functools
            prev = (0, *ps0)
            for j in range(1, JT):
                mt = load_mask(j)
                pj, pA, pB = prev
                mids = {
                    2: functools.partial(fin_a, pj, pA, pB),
                    8: functools.partial(fin_b, pj),
                }
                if j == 1:
                    mids[14] = functools.partial(pre_xpart, 6)
                    mids[20] = functools.partial(pre_xpart, 7)
                ps = stage1(j, mt, mids)
                prev = (j, *ps)
            fin_a(*prev)
            fin_b(prev[0])

    return nc


_cached = {}


def _get_bass(with_bias: bool):
    if with_bias not in _cached:
        _cached[with_bias] = build_bass(with_bias)
    return _cached[with_bias]


def _host_prep(x, adj, weight, bias):
    import ml_dtypes

    fp8 = ml_dtypes.float8_e4m3
    bf16 = ml_dtypes.bfloat16
    x = np.asarray(x, dtype=np.float32)
    adj = np.asarray(adj)
    weight = np.asarray(weight, dtype=np.float32)
    bias = np.asarray(bias, dtype=np.float32)

    with_bias = bool(np.any(bias))

    # replicated: [1 | x] in stage-1 layout [128 p][chunk c][1+F], fp8
    xpf = np.empty((N, FP), dtype=np.float32)
    xpf[:, 0] = 1.0 / 64.0
    xpf[:, 1:] = x
    # [p][cp][f][i]: pair elements (i = which 128-block of the 256-chunk)
    # adjacent in SBUF so the PE's DoubleRow moving fetch reads 2 B/column
    xp_t = np.ascontiguousarray(
        xpf.reshape(CP, 2, 128, FP).transpose(2, 0, 3, 1)
    ).reshape(128, CP * FP * 2).astype(fp8)

    # stage-2 weights: x1-part (f < 256) fp8 with pairs interleaved,
    # w1f8[p, k*O+o, i] = W[k, i*128+p, o]; x-part bf16, k=0,1 fused
    w1f8_t = np.ascontiguousarray(
        weight[:, 0:F, :].reshape(3, 2, 128, O).transpose(2, 0, 3, 1)
    ).reshape(128, 3 * O * 2).astype(fp8)
    w_rx = weight[:, F:2 * F, :].reshape(3, 2, 128, O)
    w01x_t = np.ascontiguousarray(
        w_rx[0:2].transpose(1, 2, 0, 3)
    ).reshape(2, 128, 2 * O).astype(bf16)
    w2x_t = np.ascontiguousarray(w_rx[2]).astype(bf16)
    bias_r = np.broadcast_to(bias, (128, O)).copy() if with_bias else None

    mask = (adj == 1)
    in_maps = []
    for c in range(NCORES):
        rows = slice(c * NB, (c + 1) * NB)
        # adjT shard in layout [j][p][c][n]: element
        # [j, p, c*128 + n] = mask[node j*128+n, m=c*128+p], fp8
        a = mask[rows].T.astype(fp8)                   # [N m, NB n]
        a = a.reshape(CP, 2, 128, JT, 128)             # [cp, i, p, j, n]
        a = np.ascontiguousarray(a.transpose(3, 2, 0, 1, 4)).reshape(
            JT, 128, CP * 128 * 2
        )
        xt_c = np.ascontiguousarray(64.0 * x[rows].T).reshape(
            F // 128, 128, NB).astype(bf16)
        m = {"maskt": a, "xp": xp_t, "xt": xt_c, "w1f8": w1f8_t,
             "w01x": w01x_t, "w2x": w2x_t}
        if with_bias:
            m["biasr"] = bias_r
        in_maps.append(m)
    return in_maps, with_bias


def run(x, adj, weight, bias, trace=False, trace_kwargs=None):
    """Shard, run on 8 cores, gather. Returns (out_full, BassKernelResults)."""
    from concourse.bass_utils import run_bass_kernel_spmd

    in_maps, with_bias = _host_prep(x, adj, weight, bias)
    nc = _get_bass(with_bias)
    res = run_bass_kernel_spmd(
        nc, in_maps, list(range(NCORES)), trace=trace, **(trace_kwargs or {})
    )
    out_full = np.empty((3, N, O), dtype=np.float32)
    for c in range(NCORES):
        o = np.asarray(res.results[c]["out"], dtype=np.float32)
        out_full[:, c * NB:(c + 1) * NB, :] = o.reshape(
            NB, 3, O).transpose(1, 0, 2)
    return out_full, res


def kernel(g, x, adj, weight, bias):
    out, _ = run(x, adj, weight, bias)
    return out


# revision 16
# speedup vs baseline: 1.0661x; 1.0661x over previous
"""Trainium2 Bass kernel for CoreSageLayer (GNN mean-aggregate + 3-way linear).

Computation (reference):
    mask = (adj == 1)                      # [N, N] 0/1
    deg  = mask.sum(axis=1)                # [N]
    x1   = (mask @ x) / deg[:, None]       # [N, F]
    out[k] = concat([x1, x], 1) @ W[k] + bias   # [3, N, O]

Distribution: row-shard adj / x1 / out over nodes across 8 cores; replicate
x and weights; no collectives (rows independent).

Device schedule per core (nodes NB=1024, 8 node-tiles of 128):
  stage 1 (per node-tile j): fp8e4m3 DoubleRow matmuls — each instruction
      contracts a 256-neighbor chunk-pair (2 fp8 rows per PE cell, 0.5
      cycles per output column). lhsT = maskT chunk [128, 2, 128], rhs =
      [1 | x] fp8 chunk. Column 0 of the moving tensor is the ones column,
      so PSUM col 0 accumulates the exact degree (0/1 products in fp32).
      Output columns split 129 + 128 across two PSUM tiles because the
      DoubleRow moving limit is 512 (2*257 = 514 would exceed it).
  finalize j: rec = 1/deg (DVE), x1 = psum * rec -> bf16, PE-transpose x1
      into x1T (bf16), then stage 2: out[k] = [x1 | x] @ W[k] as bf16
      matmuls with W for k=0,1 fused into one 512-wide moving tensor.
      Results are written back as bf16 (cast to f32 on host).
"""

import sys

sys.path.insert(0, "/opt/trn_rl_repo")

import numpy as np

N = 8192
F = 256
O = 256
NCORES = 8
NB = N // NCORES          # nodes per core (1024)
JT = NB // 128            # node tiles per core (8)
MCHUNKS = N // 128        # contraction chunks of 128 (64)
CP = MCHUNKS // 2         # chunk-pairs of 256 for DoubleRow (32)
FP = F + 1                # ones column + x columns (257)


def _patch_tile_drain():
    """This container's walrus allows only one sync-wait per CTRL instruction;
    split the Tile kernel-tail drain's waits onto single-wait no-fuse NoOps."""
    import concourse.tile as tile
    from concourse import mybir
    from concourse.tile import ScopedClock

    if getattr(tile.TileContext, "_drain_split_patched", False):
        return

    def _drain_and_barrier(self, tick_clock, wait_clock):
        nc = self.nc
        drain_inst = nc.sync.drain()
        wait_clock.add_sem_waits(
            drain_inst.ins, ScopedClock({None: tick_clock.global_clock})
        )
        si = drain_inst.ins.sync_info
        if si is not None and len(si.on_wait) > 1:
            waits = list(si.on_wait)
            drain_inst.ins.sync_info = mybir.SyncInfo(
                on_wait=[waits[0]], on_update=list(si.on_update)
            )
            for w in waits[1:]:
                nop = nc.sync.nop(nofuse=True, hint="split_wait")
                nop.ins.sync_info = mybir.SyncInfo(on_wait=[w], on_update=[])
        nc.all_engine_barrier()
        assert self.sems is not None
        popped = nc._tile_sem_poison_stack.pop()
        assert popped is self._sem_poison
        nc.clear_and_free_semaphores(list(self.sems.allocated().values()))
        nc.all_engine_barrier()

    tile.TileContext._drain_and_barrier = _drain_and_barrier
    tile.TileContext._drain_split_patched = True

    # Same walrus limitation, general case: any instruction that Tile gave
    # >1 sem-wait (e.g. a DMA with both RAW and WAR deps) fails codegen.
    # Split surplus waits onto fresh single-wait NoOps emitted just before
    # the instruction on the same engine, at the serialized-BIR level.
    import concourse.bass as bass
    import orjson

    _orig_to_json_bytes = bass.Bass.to_json_bytes

    def _to_json_bytes_split(self):
        m = orjson.loads(_orig_to_json_bytes(self))
        ctr = 0
        for fn in m.get("functions", []):
            for bb in fn.get("blocks", []):
                insts = bb.get("instructions", [])
                # Dedupe redundant PE weight loads: legalization emits one
                # Ldweights per Matmult, but consecutive matmuls that share
                # a stationary tensor (the two output-column groups per mask
                # chunk-pair; the three k's per stage-2 f-chunk) only need
                # the first — the PE array keeps weights across matmuls.
                # A duplicate with sync waits/updates becomes a NoOp that
                # preserves them; a bare one is dropped.
                deduped = []
                cur_key = None
                for inst in insts:
                    if inst.get("engine") != "PE":
                        deduped.append(inst)
                        continue
                    op = inst.get("opcode")
                    if op == "Ldweights":
                        key = orjson.dumps([
                            inst.get("ins"), inst.get("perf_mode"),
                            inst.get("is_transpose"),
                            inst.get("tile_position"), inst.get("tile_size"),
                        ])
                        if key == cur_key:
                            si = inst.get("sync_info")
                            if si and (si.get("on_wait") or si.get("on_update")):
                                deduped.append({
                                    "name": inst["name"] + "-LDWNOP",
                                    "opcode": "NoOp",
                                    "engine": "PE",
                                    "ins": [],
                                    "outs": [],
                                    "sync_info": si,
                                })
                            continue
                        cur_key = key
                    elif op != "Matmult":
                        cur_key = None
                    deduped.append(inst)
                insts = deduped
                new = []
                for inst in insts:
                    si = inst.get("sync_info")
                    waits = (si or {}).get("on_wait") or []
                    if len(waits) > 1:
                        for w in waits[:-1]:
                            ctr += 1
                            new.append({
                                "name": f"SWNOP-{ctr}",
                                "opcode": "NoOp",
                                "engine": inst["engine"],
                                "ins": [],
                                "outs": [],
                                "sync_info": {"on_wait": [w], "on_update": []},
                            })
                        si["on_wait"] = [waits[-1]]
                    new.append(inst)
                bb["instructions"] = new
        return orjson.dumps(m)

    bass.Bass.to_json_bytes = _to_json_bytes_split



def build_bass(with_bias: bool):
    import concourse.bass as bass
    import concourse.tile as tile
    from concourse import mybir
    from concourse.masks import make_identity

    _patch_tile_drain()

    fp8 = mybir.dt.float8e4
    bf16 = mybir.dt.bfloat16
    f32 = mybir.dt.float32
    DR = mybir.MatmulPerfMode.DoubleRow

    nc = bass.Bass()
    maskt = nc.dram_tensor("maskt", [JT, 128, CP * 128 * 2], fp8,
                           kind="ExternalInput")
    xp = nc.dram_tensor("xp", [128, CP * FP * 2], fp8, kind="ExternalInput")
    xt = nc.dram_tensor("xt", [F // 128, 128, NB], bf16, kind="ExternalInput")
    # x1-part weights (f < 256) as fp8 pairs for the DoubleRow contraction;
    # x-part weights (f >= 256) in bf16, k=0,1 fused along the output dim
    w1f8 = nc.dram_tensor("w1f8", [128, 3 * O * 2], fp8, kind="ExternalInput")
    w01x = nc.dram_tensor("w01x", [2, 128, 2 * O], bf16, kind="ExternalInput")
    w2x = nc.dram_tensor("w2x", [2, 128, O], bf16, kind="ExternalInput")
    if with_bias:
        biasr = nc.dram_tensor("biasr", [128, O], f32, kind="ExternalInput")
    out = nc.dram_tensor("out", [JT, 128, 3 * O], bf16, kind="ExternalOutput")

    FCH = 2 * F // 128  # 4 f-chunks of 128 in the stage-2 contraction

    with tile.TileContext(nc) as tc:
        with (
            tc.tile_pool(name="const", bufs=1) as const_pool,
            tc.tile_pool(name="mask", bufs=8) as mask_pool,
            tc.tile_pool(name="work", bufs=3) as work_pool,
            tc.tile_pool(name="psumS", bufs=4, space="PSUM") as psumS_pool,
            tc.tile_pool(name="psumt", bufs=2, space="PSUM") as psumt_pool,
            tc.tile_pool(name="psum2", bufs=2, space="PSUM") as psum2_pool,
        ):
            # ---- DMA plan: three HWDGE queues.
            #   sync   : mask j0 (pieces), j1, j2, j4, j6
            #   scalar : stage-2 constants, then mask j3, j5, j7
            #   gpsimd : [1|x] pieces, then per-tile output writes
            # First pieces are small so the PE unblocks right after the
            # queues come up.
            mt0 = mask_pool.tile([128, CP, 2, 128], fp8, tag="mt", name="mt0")
            xp_sb = const_pool.tile([128, CP, FP, 2], fp8)
            # Head race: per-queue HWDGE throughput is ~150-180 GB/s, well
            # below the ~400+ GB/s three-queue aggregate. Everything the PE
            # needs in the first ~20us (xp, mask j0/j1, stage-2 consts) is
            # cut into pieces and round-robined across all three queues in
            # deadline order.
            _rr = [nc.sync, nc.gpsimd, nc.scalar]
            _rrn = [0]

            def rr_dma(dst, src):
                q = _rr[_rrn[0] % 3]
                _rrn[0] += 1
                q.dma_start(dst, src)

            # stage-2 constants are allocated up front: the x-part
            # precompute (which fills PE stall bubbles during the head DMA
            # race) needs xT and the x-part weights within the first ~15us.
            w1f8_sb = const_pool.tile([128, 3 * O, 2], fp8)
            xt_sb = [const_pool.tile([128, NB], bf16, tag=f"xt{h}", name=f"xt{h}")
                     for h in range(2)]
            w01x_sb = [const_pool.tile([128, 2 * O], bf16, tag=f"w01x_{fc}",
                                       name=f"w01x_{fc}") for fc in range(2)]
            w2x_sb = [const_pool.tile([128, O], bf16, tag=f"w2x_{fc}",
                                      name=f"w2x_{fc}") for fc in range(2)]

            PIECES = [1, 1, 2, 2, 2, 4, 4, 8, 8]
            offs = []
            off = 0
            for w in PIECES:
                offs.append((off, w))
                off += w
            for n, (off, w) in enumerate(offs):
                rr_dma(xp_sb[:, off:off + w, :, :],
                       xp[:, off * FP * 2:(off + w) * FP * 2])
                rr_dma(mt0[:, off:off + w, :, :],
                       maskt[0, :, off * 256:(off + w) * 256])
                if n == 4:
                    rr_dma(xt_sb[0][:], xt[0])
                elif n == 5:
                    rr_dma(xt_sb[1][:], xt[1])
                    rr_dma(w01x_sb[0][:], w01x[0])
                elif n == 6:
                    rr_dma(w01x_sb[1][:], w01x[1])
                    rr_dma(w2x_sb[0][:], w2x[0])
                    rr_dma(w2x_sb[1][:], w2x[1])
            rr_dma(w1f8_sb[:], w1f8[:])

            def stage1(j, mt, mids):
                # one PSUM bank per tile: columns [0] deg/64, [1:129] x1
                # low half, [129:257] x1 high half. The first matmul's
                # start flag zeroes the whole bank (bank-granular on this
                # HW), so the B-group accumulates with start=False.
                ps = psumS_pool.tile([128, FP], f32, tag="ps")
                for c in range(CP):
                    if c in mids:
                        mids[c]()
                    lhsT = mt[:, c, :, :]
                    nc.tensor.matmul(
                        ps[:, 0:129], lhsT,
                        xp_sb[:, c, 0:129, :].rearrange("p f i -> p i f"),
                        start=(c == 0), stop=(c == CP - 1), perf_mode=DR,
                        skip_group_check=True,
                    )
                    nc.tensor.matmul(
                        ps[:, 129:257], lhsT,
                        xp_sb[:, c, 129:257, :].rearrange("p f i -> p i f"),
                        start=False, stop=(c == CP - 1), perf_mode=DR,
                        skip_group_check=True,
                    )
                return ps

            ps_tiles = {}
            ps_tiles[0] = stage1(0, mt0, {})

            if with_bias:
                bias_sb = const_pool.tile([128, O], f32)
            identity = const_pool.tile([128, 128], bf16)
            make_identity(nc, identity)
            # stage-2 runs on 64*x1 / 64*x (keeps x1 out of fp8's subnormal
            # range); the output copy folds the 1/64 back in
            inv64 = const_pool.tile([128, 1], f32)
            nc.gpsimd.memset(inv64[:], 1.0 / 64.0)
            # x1T planes: x1t_sb[p, i, n] = x1[n, i*128 + p], fp8 for the
            # DoubleRow stage-2 contraction (stationary planes separated)
            x1t_sb = const_pool.tile([128, 2, NB], fp8)

            def load_mask(j):
                mt = mask_pool.tile([128, CP, 2, 128], fp8, tag="mt",
                                    name=f"mt{j}")
                npieces = 4 if j <= 3 else 2
                w = CP // npieces
                for qn in range(npieces):
                    rr_dma(mt[:, qn * w:(qn + 1) * w, :, :],
                           maskt[j, :, qn * w * 256:(qn + 1) * w * 256])
                return mt

            def load_consts():
                # emitted after mask j3: the queues deliver xp + masks
                # j0-j3 first, these land by the time finalize(0) runs
                # (injected into stage1(3))
                rr_dma(w1f8_sb[:], w1f8[:])
                for h in range(2):
                    rr_dma(xt_sb[h][:], xt[h])
                for fc in range(2):
                    rr_dma(w01x_sb[fc][:], w01x[fc])
                for fc in range(2):
                    rr_dma(w2x_sb[fc][:], w2x[fc])
                if with_bias:
                    rr_dma(bias_sb[:], biasr[:])

            def fin_a(j):
                # 1/deg scale + transpose x1 into fp8 planes
                ps = ps_tiles[j]
                jcols = slice(j * 128, (j + 1) * 128)
                rec = work_pool.tile([128, 1], f32, tag="rec")
                nc.vector.reciprocal(rec[:], ps[:, 0:1])
                x1h = [work_pool.tile([128, 128], bf16, tag=f"x1h{h}",
                                      name=f"x1h{h}") for h in range(2)]
                nc.vector.tensor_scalar_mul(x1h[0][:], ps[:, 1:129], rec[:])
                nc.vector.tensor_scalar_mul(x1h[1][:], ps[:, 129:257], rec[:])
                for h in range(2):
                    pt = psumt_pool.tile([128, 128], bf16, tag="pt")
                    nc.tensor.transpose(pt[:], x1h[h][:], identity[:])
                    nc.vector.tensor_copy(x1t_sb[:, h, jcols], pt[:])

            def fin_b(j):
                # stage 2: out[k] = x1 @ W1[k] (fp8 DoubleRow, one weight
                # load for all three k) + x @ W2[k] (bf16) + bias.
                # The full-width x-part matmul opens each PSUM bank.
                jcols = slice(j * 128, (j + 1) * 128)
                po01 = psum2_pool.tile([128, 2 * O], f32, tag="po")
                po2 = psum2_pool.tile([128, 2 * O], f32, tag="po")
                lhsDR = x1t_sb[:, :, jcols]

                def w1k(k):
                    return w1f8_sb[:, k * O:(k + 1) * O, :].rearrange(
                        "p o i -> p i o")

                nc.tensor.matmul(po01[:], xt_sb[0][:, jcols], w01x_sb[0][:],
                                 start=True, stop=False, skip_group_check=True)
                nc.tensor.matmul(po2[:, 0:O], xt_sb[0][:, jcols],
                                 w2x_sb[0][:], start=True, stop=False,
                                 skip_group_check=True)
                targets = [(po01, 0), (po01, O), (po2, 0)]
                for k, (po, o0) in enumerate(targets):
                    nc.tensor.matmul(
                        po[:, o0:o0 + O], lhsDR, w1k(k),
                        start=False, stop=False, perf_mode=DR,
                        skip_group_check=True,
                    )
                nc.tensor.matmul(po01[:], xt_sb[1][:, jcols], w01x_sb[1][:],
                                 start=False, stop=True, skip_group_check=True)
                nc.tensor.matmul(po2[:, 0:O], xt_sb[1][:, jcols],
                                 w2x_sb[1][:], start=False, stop=True,
                                 skip_group_check=True)
                ot = work_pool.tile([128, 3 * O], bf16, tag="ot")
                if with_bias:
                    sc = work_pool.tile([128, 3 * O], f32, tag="sc")
                    nc.vector.tensor_scalar_mul(sc[:, 0:2 * O], po01[:],
                                                inv64[:])
                    nc.vector.tensor_scalar_mul(sc[:, 2 * O:], po2[:, 0:O],
                                                inv64[:])
                    nc.vector.tensor_add(ot[:, 0:O], sc[:, 0:O], bias_sb[:])
                    nc.vector.tensor_add(ot[:, O:2 * O], sc[:, O:2 * O],
                                         bias_sb[:])
                    nc.vector.tensor_add(ot[:, 2 * O:], sc[:, 2 * O:],
                                         bias_sb[:])
                else:
                    nc.vector.tensor_scalar_mul(ot[:, 0:2 * O], po01[:],
                                                inv64[:])
                    nc.vector.tensor_scalar_mul(ot[:, 2 * O:], po2[:, 0:O],
                                                inv64[:])
                nc.gpsimd.dma_start(out[j], ot[:])

            # Deep software pipeline: finalize(j) runs three tiles later,
            # inside stage1(j+3)'s chunk loop, so the stage-2 constants are
            # not needed until the head DMA race (xp + early masks) is over.
            # stage1(6)/stage1(7) each absorb two finalizes; only tile 7's
            # finalize remains after the last chunk.
            import functools as _f
            FINS = {3: [0], 4: [1], 5: [2], 6: [3, 4], 7: [5, 6]}
            for j in range(1, JT):
                mt = load_mask(j)
                if j == 3:
                    load_consts()
                mids = {}
                fl = FINS.get(j, [])
                if len(fl) >= 1:
                    mids[2] = _f.partial(fin_a, fl[0])
                    mids[8] = _f.partial(fin_b, fl[0])
                if len(fl) == 2:
                    mids[18] = _f.partial(fin_a, fl[1])
                    mids[24] = _f.partial(fin_b, fl[1])
                ps_tiles[j] = stage1(j, mt, mids)
            fin_a(7)
            fin_b(7)

    return nc


_cached = {}


def _get_bass(with_bias: bool):
    if with_bias not in _cached:
        _cached[with_bias] = build_bass(with_bias)
    return _cached[with_bias]


def _host_prep(x, adj, weight, bias):
    import ml_dtypes

    fp8 = ml_dtypes.float8_e4m3
    bf16 = ml_dtypes.bfloat16
    x = np.asarray(x, dtype=np.float32)
    adj = np.asarray(adj)
    weight = np.asarray(weight, dtype=np.float32)
    bias = np.asarray(bias, dtype=np.float32)

    with_bias = bool(np.any(bias))

    # replicated: [1 | x] in stage-1 layout [128 p][chunk c][1+F], fp8
    xpf = np.empty((N, FP), dtype=np.float32)
    xpf[:, 0] = 1.0 / 64.0
    xpf[:, 1:] = x
    # [p][cp][f][i]: pair elements (i = which 128-block of the 256-chunk)
    # adjacent in SBUF so the PE's DoubleRow moving fetch reads 2 B/column
    xp_t = np.ascontiguousarray(
        xpf.reshape(CP, 2, 128, FP).transpose(2, 0, 3, 1)
    ).reshape(128, CP * FP * 2).astype(fp8)

    # stage-2 weights: x1-part (f < 256) fp8 with pairs interleaved,
    # w1f8[p, k*O+o, i] = W[k, i*128+p, o]; x-part bf16, k=0,1 fused
    w1f8_t = np.ascontiguousarray(
        weight[:, 0:F, :].reshape(3, 2, 128, O).transpose(2, 0, 3, 1)
    ).reshape(128, 3 * O * 2).astype(fp8)
    w_rx = weight[:, F:2 * F, :].reshape(3, 2, 128, O)
    w01x_t = np.ascontiguousarray(
        w_rx[0:2].transpose(1, 2, 0, 3)
    ).reshape(2, 128, 2 * O).astype(bf16)
    w2x_t = np.ascontiguousarray(w_rx[2]).astype(bf16)
    bias_r = np.broadcast_to(bias, (128, O)).copy() if with_bias else None

    mask = (adj == 1)
    in_maps = []
    for c in range(NCORES):
        rows = slice(c * NB, (c + 1) * NB)
        # adjT shard in layout [j][p][c][n]: element
        # [j, p, c*128 + n] = mask[node j*128+n, m=c*128+p], fp8
        a = mask[rows].T.astype(fp8)                   # [N m, NB n]
        a = a.reshape(CP, 2, 128, JT, 128)             # [cp, i, p, j, n]
        a = np.ascontiguousarray(a.transpose(3, 2, 0, 1, 4)).reshape(
            JT, 128, CP * 128 * 2
        )
        xt_c = np.ascontiguousarray(64.0 * x[rows].T).reshape(
            F // 128, 128, NB).astype(bf16)
        m = {"maskt": a, "xp": xp_t, "xt": xt_c, "w1f8": w1f8_t,
             "w01x": w01x_t, "w2x": w2x_t}
        if with_bias:
            m["biasr"] = bias_r
        in_maps.append(m)
    return in_maps, with_bias


def run(x, adj, weight, bias, trace=False, trace_kwargs=None):
    """Shard, run on 8 cores, gather. Returns (out_full, BassKernelResults)."""
    from concourse.bass_utils import run_bass_kernel_spmd

    in_maps, with_bias = _host_prep(x, adj, weight, bias)
    nc = _get_bass(with_bias)
    res = run_bass_kernel_spmd(
        nc, in_maps, list(range(NCORES)), trace=trace, **(trace_kwargs or {})
    )
    out_full = np.empty((3, N, O), dtype=np.float32)
    for c in range(NCORES):
        o = np.asarray(res.results[c]["out"], dtype=np.float32)
        out_full[:, c * NB:(c + 1) * NB, :] = o.reshape(
            NB, 3, O).transpose(1, 0, 2)
    return out_full, res


def kernel(g, x, adj, weight, bias):
    out, _ = run(x, adj, weight, bias)
    return out
